# revision 18
# baseline (speedup 1.0000x reference)
"""Causal cross-attention Trainium2 kernel.

Sharding: 8 cores = 2 batches x 4 head-groups (4 heads / 256 dims each).
Per core: QKV projections (contract C=1024; x/context pre-transposed on
host), attention in transposed layout (scores [s, t] so the softmax
denominator comes free via an extra ones-column in V), causal block
skipping, per-head normalization (exact fp32), output projection
producing a partial [T, C] that the host sums over the 4 head-group
cores (+ o_b).

Matmul operands are bf16 (full PE rate); accumulation is fp32 in PSUM;
softmax normalization (reciprocal + broadcast) is exact fp32.
"""

import sys

for _p in ("/opt/trn_rl_repo",):
    if _p not in sys.path:
        sys.path.insert(0, _p)

import ml_dtypes
import numpy as np

import concourse.bacc as bacc
import concourse.mybir as mybir
import concourse.tile as tile
from concourse.bass_utils import run_bass_kernel_spmd

F32 = mybir.dt.float32
F32R = mybir.dt.float32r
BF16 = mybir.dt.bfloat16
AF = mybir.ActivationFunctionType
OP = mybir.AluOpType

B, T, S, C = 2, 2048, 2048, 1024
H, D = 16, 64
NCORES = 8
G = 4              # head groups = cores per batch
HPG = H // G       # heads per group (4)
DG = HPG * D       # 256 dims per group
KO = C // 128      # 8 contraction chunks
TCH = 512          # t-chunk width
NT = T // TCH      # 4
NSB = S // 128     # 16 s-blocks

MM_DT = BF16       # matmul operand dtype (BF16 or F32R)

_NC = None


def _np_mm_dt():
    return ml_dtypes.bfloat16 if MM_DT == BF16 else np.float32


def _build():
    nc = bacc.Bacc()
    xT = nc.dram_tensor("xT", [KO, 128, T], MM_DT, kind="ExternalInput")
    ctxT = nc.dram_tensor("ctxT", [KO, 128, S], MM_DT, kind="ExternalInput")
    qw = nc.dram_tensor("qw", [KO, 128, DG], MM_DT, kind="ExternalInput")
    kw = nc.dram_tensor("kw", [KO, 128, DG], MM_DT, kind="ExternalInput")
    vw = nc.dram_tensor("vw", [KO, 128, DG], MM_DT, kind="ExternalInput")
    ow = nc.dram_tensor("ow", [2, 128, C], MM_DT, kind="ExternalInput")
    qb = nc.dram_tensor("qb", [128, 2], F32, kind="ExternalInput")
    kb = nc.dram_tensor("kb", [128, 2], F32, kind="ExternalInput")
    vb = nc.dram_tensor("vb", [1, DG], MM_DT, kind="ExternalInput")
    tri = nc.dram_tensor("tri", [128, 128], MM_DT, kind="ExternalInput")
    ones = nc.dram_tensor("ones", [128, 128], MM_DT, kind="ExternalInput")
    y = nc.dram_tensor("y", [T, C], F32, kind="ExternalOutput")
    y_ap = y.ap()

    with tile.TileContext(nc) as tc:
        with (
            tc.tile_pool(name="const", bufs=1) as cp,
            tc.tile_pool(name="persist", bufs=1) as pp,
            tc.tile_pool(name="stream", bufs=2) as sp,
            tc.tile_pool(name="work", bufs=3) as wp,
            tc.tile_pool(name="ps", bufs=2, space="PSUM") as psp,
        ):
            qw_sb = cp.tile([128, KO, DG], MM_DT)
            kw_sb = cp.tile([128, KO, DG], MM_DT)
            vw_sb = cp.tile([128, KO, DG], MM_DT)
            ow_sb = cp.tile([128, 2, C], MM_DT)
            qb_sb = cp.tile([128, 2], F32)
            kb_sb = cp.tile([128, 2], F32)
            vb_sb = cp.tile([1, DG], MM_DT)
            tri_sb = cp.tile([128, 128], MM_DT)
            ones_sb = cp.tile([128, 128], MM_DT)
            ones_f32 = cp.tile([128, 128], F32)
            nc.scalar.dma_start(qw_sb, qw.rearrange("ko p m -> p ko m"))
            nc.scalar.dma_start(kw_sb, kw.rearrange("ko p m -> p ko m"))
            nc.scalar.dma_start(vw_sb, vw.rearrange("ko p m -> p ko m"))
            nc.scalar.dma_start(ow_sb, ow.rearrange("kb p m -> p kb m"))
            nc.scalar.dma_start(qb_sb, qb.ap())
            nc.scalar.dma_start(kb_sb, kb.ap())
            nc.scalar.dma_start(vb_sb, vb.ap())
            nc.scalar.dma_start(tri_sb, tri.ap())
            nc.scalar.dma_start(ones_sb, ones.ap())
            nc.vector.memset(ones_f32, 1.0)

            QT = pp.tile([128, 2, T], MM_DT)      # Q^T: [dout, t] per 128-block
            KT = pp.tile([128, 2, S], MM_DT)
            VP = pp.tile([128, NSB, HPG, D + 1], MM_DT)  # V + ones col per head
            YT = pp.tile([128, 2, T], MM_DT)      # normalized attention out^T
            nc.scalar.dma_start(
                VP[:, :, :, D : D + 1],
                ones.ap()[:, 0 : NSB * HPG].rearrange("p (a b) -> p a b", a=NSB)[
                    :, :, :, None])

            # ---- phase emitters (generators yield ~1-2us units so the
            # round-robin merge below keeps PE dense while ACT runs exp) ----
            def emit_proj(ci):
                t0 = ci * TCH
                sl = slice(t0, t0 + TCH)
                xt = sp.tile([128, KO, TCH], MM_DT, tag="xt", name="xt")
                nc.sync.dma_start(xt, xT.rearrange("ko p t -> p ko t")[:, :, sl])
                ct = sp.tile([128, KO, TCH], MM_DT, tag="ct", name="ct")
                nc.sync.dma_start(ct, ctxT.rearrange("ko p t -> p ko t")[:, :, sl])
                for blk in range(2):
                    ps = psp.tile([128, TCH], F32, tag="mm512", name="psq")
                    msl = slice(blk * 128, (blk + 1) * 128)
                    for ko in range(KO):
                        nc.tensor.matmul(ps, qw_sb[:, ko, msl], xt[:, ko],
                                         start=(ko == 0), stop=(ko == KO - 1))
                    nc.vector.tensor_scalar_add(QT[:, blk, sl], ps,
                                                qb_sb[:, blk : blk + 1])
                    yield
                for blk in range(2):
                    ps = psp.tile([128, TCH], F32, tag="mm512", name="psk")
                    msl = slice(blk * 128, (blk + 1) * 128)
                    for ko in range(KO):
                        nc.tensor.matmul(ps, kw_sb[:, ko, msl], ct[:, ko],
                                         start=(ko == 0), stop=(ko == KO - 1))
                    nc.vector.tensor_scalar_add(KT[:, blk, sl], ps,
                                                kb_sb[:, blk : blk + 1])
                    yield
                for s4 in range(4):
                    j = ci * 4 + s4
                    ssl = slice(s4 * 128, (s4 + 1) * 128)
                    psv = psp.tile([128, TCH], F32, tag="mm512",
                                   name="psv")[:, 0:DG]
                    for ko in range(KO):
                        nc.tensor.matmul(psv, ct[:, ko, ssl], vw_sb[:, ko],
                                         start=(ko == 0), stop=False)
                    nc.tensor.matmul(psv, ones_sb[0:1, 0:128], vb_sb,
                                     start=False, stop=True)
                    nc.vector.tensor_copy(VP[:, j, :, 0:D],
                                          psv.rearrange("p (h d) -> p h d", h=HPG))
                    yield

            def emit_attn(pair, ti):
                t0 = ti * TCH
                attps = [psp.tile([D + 1, TCH], F32, tag="attv", bufs=2,
                                  name=f"attv{pair}_{_h}")
                         for _h in range(2)]
                njs = 4 * ti + 4
                for j in range(njs):
                    s0 = j * 128
                    off = max(0, s0 - t0)
                    n = TCH - off
                    exs = []
                    for h2 in range(2):
                        base = h2 * 64
                        sps = psp.tile([128, TCH], F32, tag="scores", bufs=3,
                                       name="sps")
                        nc.tensor.matmul(
                            sps[:, :n],
                            KT[base : base + 64, pair, s0 : s0 + 128],
                            QT[base : base + 64, pair, t0 + off : t0 + TCH],
                            start=True, stop=True)
                        ex = wp.tile([128, TCH], MM_DT, tag="exp", bufs=6,
                                     name="ex")
                        nc.scalar.activation(ex[:, :n], sps[:, :n], AF.Exp,
                                             scale=0.125)
                        if j >= 4 * ti:
                            nc.vector.tensor_tensor(ex[:, 0:128], ex[:, 0:128],
                                                    tri_sb, OP.mult)
                        exs.append(ex)
                    for h2 in range(2):
                        h = pair * 2 + h2
                        nc.tensor.matmul(
                            attps[h2][:, off:TCH], VP[:, j, h, :],
                            exs[h2][:, :n],
                            start=(j == 0), stop=(j == njs - 1),
                            skip_group_check=True)
                    yield
                for h2 in range(2):
                    a = wp.tile([D + 1, TCH], F32, tag="A", bufs=2, name="a")
                    nc.vector.tensor_copy(a, attps[h2])
                    r0 = wp.tile([1, 2 * TCH], F32, tag="r0", bufs=2, name="r0")
                    nc.sync.dma_start(r0[:, 0:TCH], a[D : D + 1, 0:TCH])
                    nc.vector.reciprocal_approx_fast(
                        out=r0[:, TCH : 2 * TCH], in_=r0[:, 0:TCH])
                    bc = psp.tile([128, TCH], F32, tag="aux", bufs=1,
                                  name="bc")[0:D, :]
                    nc.tensor.matmul(bc, ones_f32[0:1, 0:D],
                                     r0[:, TCH : 2 * TCH],
                                     start=True, stop=True)
                    if h2 == 0:
                        nc.vector.tensor_tensor(YT[0:D, pair, t0 : t0 + TCH],
                                                a[0:D, :], bc, OP.mult)
                    else:
                        yn = wp.tile([D, TCH], MM_DT, tag="yn", bufs=2,
                                     name="yn")
                        nc.vector.tensor_tensor(yn, a[0:D, :], bc, OP.mult)
                        nc.sync.dma_start(YT[D:128, pair, t0 : t0 + TCH], yn)
                    yield

            def emit_oproj(tb):
                t0 = tb * 128
                yo = wp.tile([128, C], F32, tag="yo", bufs=2, name="yo")
                for cc in range(2):
                    ps = psp.tile([128, TCH], F32, tag="mm512", name="pso")
                    for k2 in range(2):
                        nc.tensor.matmul(ps, YT[:, k2, t0 : t0 + 128],
                                         ow_sb[:, k2, cc * TCH : (cc + 1) * TCH],
                                         start=(k2 == 0), stop=(k2 == 1))
                    nc.vector.tensor_copy(yo[:, cc * TCH : (cc + 1) * TCH], ps)
                nc.sync.dma_start(y_ap[t0 : t0 + 128, :], yo)
                yield

            def chain(*gens):
                for g in gens:
                    yield from g

            def drain(g):
                for _ in g:
                    pass

            def zip2(ga, gb):
                """Round-robin merge two unit generators, proportionally."""
                la, lb = list(ga), list(gb)
                # la/lb are exhausted by listing; instead interleave lazily:
                return None

            def merge(ga, gb):
                ga, gb = iter(ga), iter(gb)
                a_done = b_done = False
                while not (a_done and b_done):
                    if not a_done:
                        try:
                            next(ga)
                        except StopIteration:
                            a_done = True
                    if not b_done:
                        try:
                            next(gb)
                        except StopIteration:
                            b_done = True
                    yield

            # ---- interleaved schedule ----
            for ci in range(NT):
                drain(emit_proj(ci))
                drain(emit_attn(0, ci))
            for ti in range(NT):
                drain(emit_attn(1, ti))
                for tb in range(4 * ti, 4 * ti + 4):
                    drain(emit_oproj(tb))

    nc.finalize()
    return nc


def _get_nc():
    global _NC
    if _NC is None:
        _NC = _build()
    return _NC


def _make_in_maps(x, context, q_w, q_b, k_w, k_b, v_w, v_b, o_w, o_b):
    f = np.float32
    m = _np_mm_dt()
    tri_m = np.triu(np.ones((128, 128), dtype=m))
    ones_m = np.ones((128, 128), dtype=m)
    in_maps = []
    for cid in range(NCORES):
        b, g = cid // G, cid % G
        gs = slice(g * DG, (g + 1) * DG)
        in_maps.append({
            "xT": np.ascontiguousarray(x[b].T).reshape(KO, 128, T).astype(m),
            "ctxT": np.ascontiguousarray(context[b].T).reshape(KO, 128, S).astype(m),
            "qw": np.ascontiguousarray(q_w[:, gs]).reshape(KO, 128, DG).astype(m),
            "kw": np.ascontiguousarray(k_w[:, gs]).reshape(KO, 128, DG).astype(m),
            "vw": np.ascontiguousarray(v_w[:, gs]).reshape(KO, 128, DG).astype(m),
            "ow": np.ascontiguousarray(o_w[gs, :]).reshape(2, 128, C).astype(m),
            "qb": np.ascontiguousarray(np.asarray(q_b[gs]).reshape(2, 128).T).astype(f),
            "kb": np.ascontiguousarray(np.asarray(k_b[gs]).reshape(2, 128).T).astype(f),
            "vb": np.asarray(v_b[gs]).reshape(1, DG).astype(m),
            "tri": tri_m,
            "ones": ones_m,
        })
    return in_maps


def _gather(results, o_b):
    y = np.zeros((B, T, C), dtype=np.float32)
    for cid in range(NCORES):
        y[cid // G] += results[cid]["y"]
    y += np.asarray(o_b, dtype=np.float32)[None, None, :]
    return y


def _run(inputs, **kwargs):
    nc = _get_nc()
    in_maps = _make_in_maps(**{k: np.asarray(v) for k, v in inputs.items()})
    res = run_bass_kernel_spmd(nc, in_maps, core_ids=list(range(NCORES)), **kwargs)
    return _gather(res.results, np.asarray(inputs["o_b"])), res


def kernel(**inputs):
    y, _ = _run(inputs)
    return y


# revision 20
# speedup vs baseline: 1.1763x; 1.1763x over previous
"""Causal cross-attention Trainium2 kernel.

Sharding: 8 cores = 2 batches x 4 head-groups (4 heads / 256 dims each).
Per core: QKV projections (contract C=1024; x/context pre-transposed on
host), attention in transposed layout (scores [s, t] so the softmax
denominator comes free via an extra ones-column in V), causal block
skipping, per-head normalization (exact fp32), output projection
producing a partial [T, C] that the host sums over the 4 head-group
cores (+ o_b).

Matmul operands are bf16 (full PE rate); accumulation is fp32 in PSUM;
softmax normalization (reciprocal + broadcast) is exact fp32.
"""

import sys

for _p in ("/opt/trn_rl_repo",):
    if _p not in sys.path:
        sys.path.insert(0, _p)

import ml_dtypes
import numpy as np

import concourse.bacc as bacc
import concourse.mybir as mybir
import concourse.tile as tile
from concourse.bass_utils import run_bass_kernel_spmd

F32 = mybir.dt.float32
F32R = mybir.dt.float32r
BF16 = mybir.dt.bfloat16
AF = mybir.ActivationFunctionType
OP = mybir.AluOpType

B, T, S, C = 2, 2048, 2048, 1024
H, D = 16, 64
NCORES = 8
G = 4              # head groups = cores per batch
HPG = H // G       # heads per group (4)
DG = HPG * D       # 256 dims per group
KO = C // 128      # 8 contraction chunks
TCH = 512          # t-chunk width
NT = T // TCH      # 4
NSB = S // 128     # 16 s-blocks

MM_DT = BF16       # matmul operand dtype (BF16 or F32R)

_NC = None


def _np_mm_dt():
    return ml_dtypes.bfloat16 if MM_DT == BF16 else np.float32


def _build():
    nc = bacc.Bacc()
    xT = nc.dram_tensor("xT", [KO, 128, T], MM_DT, kind="ExternalInput")
    ctxT = nc.dram_tensor("ctxT", [KO, 128, S], MM_DT, kind="ExternalInput")
    qw = nc.dram_tensor("qw", [KO, 128, DG], MM_DT, kind="ExternalInput")
    kw = nc.dram_tensor("kw", [KO, 128, DG], MM_DT, kind="ExternalInput")
    vw = nc.dram_tensor("vw", [KO, 128, DG], MM_DT, kind="ExternalInput")
    ow = nc.dram_tensor("ow", [2, 128, C], MM_DT, kind="ExternalInput")
    qb = nc.dram_tensor("qb", [128, 2], F32, kind="ExternalInput")
    kb = nc.dram_tensor("kb", [128, 2], F32, kind="ExternalInput")
    vb = nc.dram_tensor("vb", [1, DG], MM_DT, kind="ExternalInput")
    tri = nc.dram_tensor("tri", [128, 128], MM_DT, kind="ExternalInput")
    ones = nc.dram_tensor("ones", [128, 128], MM_DT, kind="ExternalInput")
    y = nc.dram_tensor("y", [T, C], F32, kind="ExternalOutput")
    y_ap = y.ap()

    with tile.TileContext(nc) as tc:
        with (
            tc.tile_pool(name="const", bufs=1) as cp,
            tc.tile_pool(name="persist", bufs=1) as pp,
            tc.tile_pool(name="stream", bufs=2) as sp,
            tc.tile_pool(name="work", bufs=3) as wp,
            tc.tile_pool(name="ps", bufs=2, space="PSUM") as psp,
        ):
            qw_sb = cp.tile([128, KO, DG], MM_DT)
            kw_sb = cp.tile([128, KO, DG], MM_DT)
            vw_sb = cp.tile([128, KO, DG], MM_DT)
            ow_sb = cp.tile([128, 2, C], MM_DT)
            qb_sb = cp.tile([128, 2], F32)
            kb_sb = cp.tile([128, 2], F32)
            vb_sb = cp.tile([1, DG], MM_DT)
            tri_sb = cp.tile([128, 128], MM_DT)
            ones_sb = cp.tile([128, 128], MM_DT)
            ones_f32 = cp.tile([128, 128], F32)
            nc.scalar.dma_start(qw_sb, qw.rearrange("ko p m -> p ko m"))
            nc.scalar.dma_start(kw_sb, kw.rearrange("ko p m -> p ko m"))
            nc.scalar.dma_start(vw_sb, vw.rearrange("ko p m -> p ko m"))
            nc.scalar.dma_start(ow_sb, ow.rearrange("kb p m -> p kb m"))
            nc.scalar.dma_start(qb_sb, qb.ap())
            nc.scalar.dma_start(kb_sb, kb.ap())
            nc.scalar.dma_start(vb_sb, vb.ap())
            nc.scalar.dma_start(tri_sb, tri.ap())
            nc.scalar.dma_start(ones_sb, ones.ap())
            nc.vector.memset(ones_f32, 1.0)

            QT = pp.tile([128, 2, T], MM_DT)      # Q^T: [dout, t] per 128-block
            KT = pp.tile([128, 2, S], MM_DT)
            VP = pp.tile([128, NSB, HPG, D + 1], MM_DT)  # V + ones col per head
            YT = pp.tile([128, 2, T], MM_DT)      # normalized attention out^T
            nc.scalar.dma_start(
                VP[:, :, :, D : D + 1],
                ones.ap()[:, 0 : NSB * HPG].rearrange("p (a b) -> p a b", a=NSB)[
                    :, :, :, None])

            # ---- phase emitters (generators yield ~1-2us units so the
            # round-robin merge below keeps PE dense while ACT runs exp) ----
            def emit_proj(ci):
                t0 = ci * TCH
                sl = slice(t0, t0 + TCH)
                xt = sp.tile([128, KO, TCH], MM_DT, tag="xt", name="xt")
                nc.sync.dma_start(xt, xT.rearrange("ko p t -> p ko t")[:, :, sl])
                for blk in range(2):
                    ps = psp.tile([128, TCH], F32, tag="mm512", name="psq")
                    msl = slice(blk * 128, (blk + 1) * 128)
                    for ko in range(KO):
                        nc.tensor.matmul(ps, qw_sb[:, ko, msl], xt[:, ko],
                                         start=(ko == 0), stop=(ko == KO - 1))
                    nc.vector.tensor_scalar_add(QT[:, blk, sl], ps,
                                                qb_sb[:, blk : blk + 1])
                    yield
                ct = sp.tile([128, KO, TCH], MM_DT, tag="ct", name="ct")
                nc.sync.dma_start(ct, ctxT.rearrange("ko p t -> p ko t")[:, :, sl])
                for blk in range(2):
                    ps = psp.tile([128, TCH], F32, tag="mm512", name="psk")
                    msl = slice(blk * 128, (blk + 1) * 128)
                    for ko in range(KO):
                        nc.tensor.matmul(ps, kw_sb[:, ko, msl], ct[:, ko],
                                         start=(ko == 0), stop=(ko == KO - 1))
                    nc.vector.tensor_scalar_add(KT[:, blk, sl], ps,
                                                kb_sb[:, blk : blk + 1])
                    yield
                for s4 in range(4):
                    j = ci * 4 + s4
                    ssl = slice(s4 * 128, (s4 + 1) * 128)
                    psv = psp.tile([128, DG], F32, tag="aux", bufs=1,
                                   name="psv")
                    for ko in range(KO):
                        nc.tensor.matmul(psv, ct[:, ko, ssl], vw_sb[:, ko],
                                         start=(ko == 0), stop=False)
                    nc.tensor.matmul(psv, ones_sb[0:1, 0:128], vb_sb,
                                     start=False, stop=True)
                    nc.vector.tensor_copy(VP[:, j, :, 0:D],
                                          psv.rearrange("p (h d) -> p h d", h=HPG))
                    yield

            def emit_attn(pair, ti):
                t0 = ti * TCH
                attps = [psp.tile([D + 1, TCH], F32, tag="attv", bufs=2,
                                  name=f"attv{pair}_{_h}")
                         for _h in range(2)]
                njs = 4 * ti + 4
                for j in range(njs):
                    s0 = j * 128
                    off = max(0, s0 - t0)
                    n = TCH - off
                    exs = []
                    for h2 in range(2):
                        base = h2 * 64
                        sps = psp.tile([128, TCH], F32, tag="scores", bufs=3,
                                       name="sps")
                        nc.tensor.matmul(
                            sps[:, :n],
                            KT[base : base + 64, pair, s0 : s0 + 128],
                            QT[base : base + 64, pair, t0 + off : t0 + TCH],
                            start=True, stop=True)
                        ex = wp.tile([128, TCH], MM_DT, tag="exp", bufs=32,
                                     name="ex")
                        nc.scalar.activation(ex[:, :n], sps[:, :n], AF.Exp,
                                             scale=0.125)
                        if j >= 4 * ti:
                            nc.vector.tensor_tensor(ex[:, 0:128], ex[:, 0:128],
                                                    tri_sb, OP.mult)
                        exs.append(ex)
                    for h2 in range(2):
                        h = pair * 2 + h2
                        nc.tensor.matmul(
                            attps[h2][:, off:TCH], VP[:, j, h, :],
                            exs[h2][:, :n],
                            start=(j == 0), stop=(j == njs - 1),
                            skip_group_check=True)
                    yield
                for h2 in range(2):
                    a = wp.tile([D + 1, TCH], F32, tag="A", bufs=4, name="a")
                    nc.vector.tensor_copy(a, attps[h2])
                    r0 = wp.tile([1, 2 * TCH], F32, tag="r0", bufs=2, name="r0")
                    nc.sync.dma_start(r0[:, 0:TCH], a[D : D + 1, 0:TCH])
                    nc.vector.reciprocal_approx_fast(
                        out=r0[:, TCH : 2 * TCH], in_=r0[:, 0:TCH])
                    bc = psp.tile([128, TCH], F32, tag="aux", bufs=1,
                                  name="bc")[0:D, :]
                    nc.tensor.matmul(bc, ones_f32[0:1, 0:D],
                                     r0[:, TCH : 2 * TCH],
                                     start=True, stop=True)
                    if h2 == 0:
                        nc.vector.tensor_tensor(YT[0:D, pair, t0 : t0 + TCH],
                                                a[0:D, :], bc, OP.mult)
                    else:
                        yn = wp.tile([D, TCH], MM_DT, tag="yn", bufs=2,
                                     name="yn")
                        nc.vector.tensor_tensor(yn, a[0:D, :], bc, OP.mult)
                        nc.sync.dma_start(YT[D:128, pair, t0 : t0 + TCH], yn)
                    yield

            def emit_oproj(tb):
                t0 = tb * 128
                yo = wp.tile([128, C], F32, tag="yo", bufs=2, name="yo")
                for cc in range(2):
                    ps = psp.tile([128, TCH], F32, tag="mm512", name="pso")
                    for k2 in range(2):
                        nc.tensor.matmul(ps, YT[:, k2, t0 : t0 + 128],
                                         ow_sb[:, k2, cc * TCH : (cc + 1) * TCH],
                                         start=(k2 == 0), stop=(k2 == 1))
                    nc.vector.tensor_copy(yo[:, cc * TCH : (cc + 1) * TCH], ps)
                nc.sync.dma_start(y_ap[t0 : t0 + 128, :], yo)
                yield

            def chain(*gens):
                for g in gens:
                    yield from g

            def drain(g):
                for _ in g:
                    pass

            def zip2(ga, gb):
                """Round-robin merge two unit generators, proportionally."""
                la, lb = list(ga), list(gb)
                # la/lb are exhausted by listing; instead interleave lazily:
                return None

            def merge(ga, gb):
                ga, gb = iter(ga), iter(gb)
                a_done = b_done = False
                while not (a_done and b_done):
                    if not a_done:
                        try:
                            next(ga)
                        except StopIteration:
                            a_done = True
                    if not b_done:
                        try:
                            next(gb)
                        except StopIteration:
                            b_done = True
                    yield

            # ---- schedule: dense proj, then back-to-back attention
            # groups (consecutive groups pipeline on PE/ACT; only one
            # multi-matmul PSUM accumulation group is ever open), then
            # dense output projection ----
            for ci in range(NT):
                drain(emit_proj(ci))
            for ti in range(NT):
                drain(emit_attn(0, ti))
                drain(emit_attn(1, ti))
            for tb in range(16):
                drain(emit_oproj(tb))

    nc.finalize()
    return nc


def _get_nc():
    global _NC
    if _NC is None:
        _NC = _build()
    return _NC


def _make_in_maps(x, context, q_w, q_b, k_w, k_b, v_w, v_b, o_w, o_b):
    f = np.float32
    m = _np_mm_dt()
    tri_m = np.triu(np.ones((128, 128), dtype=m))
    ones_m = np.ones((128, 128), dtype=m)
    in_maps = []
    for cid in range(NCORES):
        b, g = cid // G, cid % G
        gs = slice(g * DG, (g + 1) * DG)
        in_maps.append({
            "xT": np.ascontiguousarray(x[b].T).reshape(KO, 128, T).astype(m),
            "ctxT": np.ascontiguousarray(context[b].T).reshape(KO, 128, S).astype(m),
            "qw": np.ascontiguousarray(q_w[:, gs]).reshape(KO, 128, DG).astype(m),
            "kw": np.ascontiguousarray(k_w[:, gs]).reshape(KO, 128, DG).astype(m),
            "vw": np.ascontiguousarray(v_w[:, gs]).reshape(KO, 128, DG).astype(m),
            "ow": np.ascontiguousarray(o_w[gs, :]).reshape(2, 128, C).astype(m),
            "qb": np.ascontiguousarray(np.asarray(q_b[gs]).reshape(2, 128).T).astype(f),
            "kb": np.ascontiguousarray(np.asarray(k_b[gs]).reshape(2, 128).T).astype(f),
            "vb": np.asarray(v_b[gs]).reshape(1, DG).astype(m),
            "tri": tri_m,
            "ones": ones_m,
        })
    return in_maps


def _gather(results, o_b):
    y = np.zeros((B, T, C), dtype=np.float32)
    for cid in range(NCORES):
        y[cid // G] += results[cid]["y"]
    y += np.asarray(o_b, dtype=np.float32)[None, None, :]
    return y


def _run(inputs, **kwargs):
    nc = _get_nc()
    in_maps = _make_in_maps(**{k: np.asarray(v) for k, v in inputs.items()})
    res = run_bass_kernel_spmd(nc, in_maps, core_ids=list(range(NCORES)), **kwargs)
    return _gather(res.results, np.asarray(inputs["o_b"])), res


def kernel(**inputs):
    y, _ = _run(inputs)
    return y
